# revision 18
# baseline (speedup 1.0000x reference)
"""Log-sparse attention Trainium2 kernel (self-contained).

Problem: B=2, L=S=2048, H=8, E=D=64 fp32.
  scores = q @ k^T (per b,h); masked log-sparse; A = softmax(scores/8); out = A @ v.

Strategy:
  - Shard the 16 (b,h) pairs across 8 cores, 2 pairs per core (SPMD, same
    program, different data).
  - Everything computed transposed: S^T[s,l] = K^T.T @ Q^T so that the A@V
    matmul (lhsT = V tiles in natural [s,d] layout, rhs = P^T) needs no P
    transposes. Softmax denominator comes from a ones-column appended to V.
    No max-subtraction (scores/8 ~ N(0,1); exp is safe in fp32/fp16 range).
  - The log-sparse mask is static: only 58 of 128 (s_block=128 x l_chunk=256)
    tiles per pair are nonzero, and within those only a trimmed l-window
    matters. Windows are bin-packed into [128, <=1024] PSUM groups (2 banks);
    exp + mask-multiply run once per group.
  - fp16 operands for QK and AV matmuls (1 cyc/row on PE at any moving dim,
    vs 4 for fp32), fp32 PSUM accumulation throughout. Masked positions are
    exact zeros (multiplicative 0/1 mask), matching the reference's
    exp(-1e9/8 - m) == 0 underflow.
"""
import math

import numpy as np
import ml_dtypes

import concourse.bacc as bacc
import concourse.mybir as mybir
import concourse.tile as tile
from concourse import masks as cmasks
from concourse.bass_utils import run_bass_kernel_spmd

F32 = mybir.dt.float32
F16 = mybir.dt.float16

B, L, S, H, E, D = 2, 2048, 2048, 8, 64, 64
N_CORES = 8
PAIRS_PER_CORE = 2
LCW = 256            # l-chunk width
N_LC = L // LCW      # 8
N_SB = S // 128      # 16
ALIGN = 16
SCALE = 1.0 / math.sqrt(E)


# ---------------------------------------------------------------- host tables
def _log_row_mask(index, sub_len, win_len):
    log_l = math.ceil(np.log2(sub_len))
    mask = np.zeros(win_len, dtype=np.float32)
    if (win_len // sub_len) * 2 * log_l > index:
        mask[:index + 1] = 1
    else:
        while index >= 0:
            if index - log_l + 1 < 0:
                mask[:index] = 1
                break
            mask[index - log_l + 1:index + 1] = 1
            for i in range(0, log_l):
                new_index = index - log_l + 1 - 2 ** i
                if (index - new_index) <= sub_len and new_index >= 0:
                    mask[new_index] = 1
            index -= sub_len
    return mask


def _build_tables():
    """Returns (plan, mask_packed).

    plan[lc] = list of groups; each group is a list of slot dicts
      {sb, lmin, w, goff} -- goff is the column offset inside the [128, gspan]
      psum group tile. Each bin occupies a 512-aligned half so no matmul
      window crosses a PSUM bank boundary.
    mask_packed: [NG, 128, 1024] float16, zeros in packing holes.
    """
    m = np.stack([_log_row_mask(i, S, L) for i in range(L)])  # [l, s]
    mT = m.T  # [s, l]

    plan = []
    packed_tiles = []
    for lc in range(N_LC):
        windows = []
        for sb in range(N_SB):
            sub = mT[sb * 128:(sb + 1) * 128, lc * LCW:(lc + 1) * LCW]
            cols = np.where(sub.any(axis=0))[0]
            if len(cols) == 0:
                continue
            lmin = int(cols.min() // ALIGN) * ALIGN
            lmax = int(np.ceil((cols.max() + 1) / ALIGN) * ALIGN)
            windows.append({"sb": sb, "lmin": lmin, "w": lmax - lmin})
        # the slot covering the whole chunk must exist (first AV write per lc
        # must cover all 256 columns so start=True initializes the region)
        full = [w for w in windows if w["lmin"] == 0 and w["w"] == LCW]
        assert full, f"lc={lc} has no full-window slot"
        first = full[0]
        rest = sorted((w for w in windows if w is not first),
                      key=lambda d: -d["w"])
        # greedy first-fit into 512-wide bins, `first` seeded at bin0 offset0
        bins = [[]]
        fills = [0]
        for win in [first] + rest:
            placed = False
            for bi in range(len(bins)):
                if fills[bi] + win["w"] <= 512:
                    win = dict(win, cursor=fills[bi], bin=bi)
                    bins[bi].append(win)
                    fills[bi] += win["w"]
                    placed = True
                    break
            if not placed:
                bins.append([dict(win, cursor=0, bin=len(bins))])
                fills.append(win["w"])
        # pair bins into groups of up to 2 (one [128, <=1024] psum tile)
        groups = []
        for g0 in range(0, len(bins), 2):
            slots = []
            for bi in range(g0, min(g0 + 2, len(bins))):
                for win in bins[bi]:
                    slots.append({
                        "sb": win["sb"], "lmin": win["lmin"], "w": win["w"],
                        "goff": (bi - g0) * 512 + win["cursor"],
                    })
            span = max(s2["goff"] + s2["w"] for s2 in slots)
            groups.append({"slots": slots, "span": span})
        plan.append(groups)
        for g in groups:
            t = np.zeros((128, 1024), dtype=np.float32)
            for s2 in g["slots"]:
                sb, lmin, w, goff = s2["sb"], s2["lmin"], s2["w"], s2["goff"]
                t[:, goff:goff + w] = mT[sb * 128:(sb + 1) * 128,
                                         lc * LCW + lmin:lc * LCW + lmin + w]
            packed_tiles.append(t)

    # verification: reassembled coverage == mT exactly (each nonzero in
    # exactly one window, holes zero)
    cover = np.zeros_like(mT)
    gi = 0
    for lc in range(N_LC):
        for g in plan[lc]:
            for s2 in g["slots"]:
                sb, lmin, w, goff = s2["sb"], s2["lmin"], s2["w"], s2["goff"]
                cover[sb * 128:(sb + 1) * 128,
                      lc * LCW + lmin:lc * LCW + lmin + w] = \
                    packed_tiles[gi][:, goff:goff + w]
            gi += 1
    assert np.array_equal(cover, mT), "mask packing is wrong"

    mask_packed = np.stack(packed_tiles).astype(np.float16)
    return plan, mask_packed


_PLAN, _MASK_PACKED = _build_tables()
_NG = _MASK_PACKED.shape[0]          # total groups per pair (mask shared)


# ---------------------------------------------------------------- bass kernel
def _build_nc():
    nc = bacc.Bacc("TRN2", target_bir_lowering=False, debug=False)

    # q/k are pre-cast to fp16 and pair-concatenated host-side: [L, 128] with
    # columns 0:64 = pair0's E dims, 64:128 = pair1's. One hardware DMA
    # transpose (xbar mode, 2-byte dtype) then yields the [128, L] layout with
    # each pair on its own partition range -- no PE transposes needed.
    q_d = nc.dram_tensor("q", [L, 2 * E], F16, kind="ExternalInput").ap()
    k_d = nc.dram_tensor("k", [S, 2 * E], F16, kind="ExternalInput").ap()
    v_d = nc.dram_tensor("v", [PAIRS_PER_CORE, S, D], F16,
                         kind="ExternalInput").ap()
    mask_d = nc.dram_tensor("mask_packed", [_NG, 128, 1024], F16,
                            kind="ExternalInput").ap()
    o_d = nc.dram_tensor("o", [PAIRS_PER_CORE, L, D], F32,
                         kind="ExternalOutput").ap()

    EXP = mybir.ActivationFunctionType.Exp

    with tile.TileContext(nc) as tc:
        with (
            tc.tile_pool(name="const", bufs=1) as const_pool,
            tc.tile_pool(name="sb", bufs=1) as sb,
            tc.tile_pool(name="pp", bufs=6) as pp,
            tc.tile_pool(name="ob", bufs=2) as ob,
            tc.tile_pool(name="qk_ps", bufs=2, space="PSUM") as qk_ps,
            tc.tile_pool(name="o_ps", bufs=2, space="PSUM") as o_ps,
            tc.tile_pool(name="misc_ps", bufs=1, space="PSUM") as misc_ps,
        ):
            # Q^T / K^T via hardware DMA transpose (fp16), pair-packed on
            # partitions (pair0 -> 0:64, pair1 -> 64:128). Split in halves and
            # ordered by first consumption (early l-chunks only touch the low
            # s/l columns), all on the sync queue in that order.
            qT = const_pool.tile([128, L], F16)
            kT = const_pool.tile([128, S], F16)
            mask_sb = const_pool.tile([128, _NG, 1024], F16)
            v_sb = [const_pool.tile([128, N_SB, D + 1], F16, name=f"v_sb{p}")
                    for p in range(PAIRS_PER_CORE)]

            # q/k transposes on the sync HWDGE; v + masks on the scalar HWDGE
            # (the only other hw-DGE engine) so the two streams don't
            # serialize on shared descriptor rings.
            nc.sync.dma_start_transpose(out=kT[:, 0:1024], in_=k_d[0:1024, :])
            nc.sync.dma_start_transpose(out=qT[:, 0:1024], in_=q_d[0:1024, :])
            nc.sync.dma_start_transpose(out=kT[:, 1024:2048],
                                        in_=k_d[1024:2048, :])
            nc.sync.dma_start_transpose(out=qT[:, 1024:2048],
                                        in_=q_d[1024:2048, :])
            nc.scalar.dma_start(v_sb[0][:, :, 0:D],
                                v_d[0].rearrange("(t p) d -> p t d", p=128))
            nc.scalar.dma_start(mask_sb[:, 0:3, :],
                                mask_d[0:3].rearrange("g p c -> p g c"))
            nc.scalar.dma_start(v_sb[1][:, :, 0:D],
                                v_d[1].rearrange("(t p) d -> p t d", p=128))
            for a, b in ((3, 8), (8, _NG)):
                nc.scalar.dma_start(mask_sb[:, a:b, :],
                                    mask_d[a:b].rearrange("g p c -> p g c"))
            for p in range(PAIRS_PER_CORE):
                nc.vector.memset(v_sb[p][:, :, D], 1.0)

            ident = const_pool.tile([128, 128], F32)
            cmasks.make_identity(nc, ident)

            # zero the two QK psum group slots (4 banks) once with PE matmuls
            # so exp() of packing holes sees exp(0), never leftover inf/nan.
            # PE engine order serializes these against all later PSUM writers.
            zrow = const_pool.tile([1, 512], F16)
            nc.vector.memset(zrow, 0.0)
            for z in range(2):
                zt = qk_ps.tile([128, 1024], F32, tag="s_ps", name=f"zt{z}")
                for b in range(2):
                    nc.tensor.matmul(zt[:, b * 512:(b + 1) * 512],
                                     zrow[:, 0:128], zrow,
                                     start=True, stop=True)

            # main loop
            for p in range(PAIRS_PER_CORE):
                bp = 64 * p
                out_all = sb.tile([128, N_LC * 2, 65], F32, name=f"out_all{p}")
                for lcp in (range(N_LC // 2) if p == 0
                            else list(range(N_LC // 2 - 1, -1, -1))):
                    o_acc = o_ps.tile([65, 2 * LCW], F32, tag="o_acc")
                    for half in range(2):
                        lc = 2 * lcp + half
                        av_list = []
                        for gi, grp in enumerate(_PLAN[lc]):
                            g_idx = sum(len(_PLAN[x]) for x in range(lc)) + gi
                            span = grp["span"]
                            s_ps = qk_ps.tile([128, 1024], F32, tag="s_ps")
                            for s2 in grp["slots"]:
                                sb_, lmin, w, goff = (s2["sb"], s2["lmin"],
                                                      s2["w"], s2["goff"])
                                nc.tensor.matmul(
                                    s_ps[:, goff:goff + w],
                                    kT[bp:bp + 64, sb_ * 128:(sb_ + 1) * 128],
                                    qT[bp:bp + 64,
                                       lc * LCW + lmin:lc * LCW + lmin + w],
                                    start=True, stop=True)
                            p_t = pp.tile([128, 1024], F16, tag="p_t")
                            nc.scalar.activation(p_t[:, :span], s_ps[:, :span],
                                                 EXP, scale=SCALE)
                            nc.vector.tensor_mul(p_t[:, :span], p_t[:, :span],
                                                 mask_sb[:, g_idx, :span])
                            av_list.append((grp, p_t))
                        n_av = sum(len(g["slots"]) for g, _ in av_list)
                        ai = 0
                        for grp, p_t in av_list:
                            for s2 in grp["slots"]:
                                sb_, lmin, w, goff = (s2["sb"], s2["lmin"],
                                                      s2["w"], s2["goff"])
                                nc.tensor.matmul(
                                    o_acc[:, half * LCW + lmin:
                                          half * LCW + lmin + w],
                                    v_sb[p][:, sb_, :],
                                    p_t[:, goff:goff + w],
                                    start=(ai == 0), stop=(ai == n_av - 1),
                                    skip_group_check=True)
                                ai += 1
                    # output stage for these two l-chunks (4 l-blocks of 128)
                    o_sb = ob.tile([65, 2 * LCW], F32, tag="o_sb")
                    nc.scalar.copy(o_sb, o_acc)
                    ot = misc_ps.tile([128, 4 * 65], F32, tag="outt")
                    for j in range(4):
                        nc.tensor.transpose(ot[:, j * 65:(j + 1) * 65],
                                            o_sb[:, j * 128:(j + 1) * 128],
                                            ident[0:65, 0:65])
                    nc.vector.tensor_copy(out_all[:, 4 * lcp:4 * lcp + 4, :],
                                          ot.rearrange("p (j c) -> p j c", c=65))
                # normalize and store
                rinv = sb.tile([128, N_LC * 2], F32, name=f"rinv{p}")
                nc.vector.reciprocal(rinv, out_all[:, :, 64])
                o_n = sb.tile([128, N_LC * 2, 64], F32, name=f"o_n{p}")
                nc.vector.tensor_tensor(
                    o_n, out_all[:, :, 0:64],
                    rinv[:, :, None].to_broadcast((128, N_LC * 2, 64)),
                    mybir.AluOpType.mult)
                nc.sync.dma_start(
                    o_d[p].rearrange("(t p) d -> p t d", p=128), o_n)

    nc.finalize()
    return nc


_NC_CACHE = None


def _get_nc():
    global _NC_CACHE
    if _NC_CACHE is None:
        _NC_CACHE = _build_nc()
    return _NC_CACHE


# ------------------------------------------------------------------- wrapper
def kernel(queries, keys, values):
    queries = np.ascontiguousarray(np.asarray(queries, dtype=np.float32))
    keys = np.ascontiguousarray(np.asarray(keys, dtype=np.float32))
    values = np.ascontiguousarray(np.asarray(values, dtype=np.float32))

    in_maps = []
    for c in range(N_CORES):
        qs, ks, vs = [], [], []
        for j in range(PAIRS_PER_CORE):
            i = c * PAIRS_PER_CORE + j
            b, h = divmod(i, H)
            qs.append(queries[b, :, h, :])
            ks.append(keys[b, :, h, :])
            vs.append(values[b, :, h, :])
        in_maps.append({
            "q": np.concatenate(qs, axis=1).astype(np.float16),
            "k": np.concatenate(ks, axis=1).astype(np.float16),
            "v": np.ascontiguousarray(np.stack(vs)).astype(np.float16),
            "mask_packed": _MASK_PACKED,
        })

    res = run_bass_kernel_spmd(_get_nc(), in_maps,
                               core_ids=list(range(N_CORES)))

    out = np.empty((B, L, H, D), dtype=np.float32)
    for c in range(N_CORES):
        for j in range(PAIRS_PER_CORE):
            i = c * PAIRS_PER_CORE + j
            b, h = divmod(i, H)
            out[b, :, h, :] = res.results[c]["o"][j]
    return out


# revision 21
# speedup vs baseline: 1.2393x; 1.2393x over previous
"""Log-sparse attention Trainium2 kernel (self-contained).

Problem: B=2, L=S=2048, H=8, E=D=64 fp32.
  scores = q @ k^T (per b,h); masked log-sparse; A = softmax(scores/8); out = A @ v.

Strategy:
  - Shard the 16 (b,h) pairs across 8 cores, 2 pairs per core (SPMD, same
    program, different data).
  - Everything computed transposed: S^T[s,l] = K^T.T @ Q^T so that the A@V
    matmul (lhsT = V tiles in natural [s,d] layout, rhs = P^T) needs no P
    transposes. Softmax denominator comes from a ones-column appended to V.
    No max-subtraction (scores/8 ~ N(0,1); exp is safe in fp32/fp16 range).
  - The log-sparse mask is static: only 58 of 128 (s_block=128 x l_chunk=256)
    tiles per pair are nonzero, and within those only a trimmed l-window
    matters. Windows are bin-packed into [128, <=1024] PSUM groups (2 banks);
    exp + mask-multiply run once per group.
  - fp16 operands for QK and AV matmuls (1 cyc/row on PE at any moving dim,
    vs 4 for fp32), fp32 PSUM accumulation throughout. Masked positions are
    exact zeros (multiplicative 0/1 mask), matching the reference's
    exp(-1e9/8 - m) == 0 underflow.
"""
import math

import numpy as np
import ml_dtypes

import concourse.bacc as bacc
import concourse.mybir as mybir
import concourse.tile as tile
from concourse import masks as cmasks
from concourse.bass_utils import run_bass_kernel_spmd

F32 = mybir.dt.float32
F16 = mybir.dt.float16

B, L, S, H, E, D = 2, 2048, 2048, 8, 64, 64
N_CORES = 8
PAIRS_PER_CORE = 2
LCW = 256            # l-chunk width
N_LC = L // LCW      # 8
N_SB = S // 128      # 16
ALIGN = 16
SCALE = 1.0 / math.sqrt(E)


# ---------------------------------------------------------------- host tables
def _log_row_mask(index, sub_len, win_len):
    log_l = math.ceil(np.log2(sub_len))
    mask = np.zeros(win_len, dtype=np.float32)
    if (win_len // sub_len) * 2 * log_l > index:
        mask[:index + 1] = 1
    else:
        while index >= 0:
            if index - log_l + 1 < 0:
                mask[:index] = 1
                break
            mask[index - log_l + 1:index + 1] = 1
            for i in range(0, log_l):
                new_index = index - log_l + 1 - 2 ** i
                if (index - new_index) <= sub_len and new_index >= 0:
                    mask[new_index] = 1
            index -= sub_len
    return mask


def _build_tables():
    """Returns (plan, mask_packed).

    plan[lc] = list of groups; each group is a list of slot dicts
      {sb, lmin, w, goff} -- goff is the column offset inside the [128, gspan]
      psum group tile. Each bin occupies a 512-aligned half so no matmul
      window crosses a PSUM bank boundary.
    mask_packed: [NG, 128, 1024] float16, zeros in packing holes.
    """
    m = np.stack([_log_row_mask(i, S, L) for i in range(L)])  # [l, s]
    mT = m.T  # [s, l]

    plan = []
    packed_tiles = []
    for lc in range(N_LC):
        windows = []
        for sb in range(N_SB):
            sub = mT[sb * 128:(sb + 1) * 128, lc * LCW:(lc + 1) * LCW]
            cols = np.where(sub.any(axis=0))[0]
            if len(cols) == 0:
                continue
            lmin = int(cols.min() // ALIGN) * ALIGN
            lmax = int(np.ceil((cols.max() + 1) / ALIGN) * ALIGN)
            windows.append({"sb": sb, "lmin": lmin, "w": lmax - lmin})
        # the slot covering the whole chunk must exist (first AV write per lc
        # must cover all 256 columns so start=True initializes the region)
        full = [w for w in windows if w["lmin"] == 0 and w["w"] == LCW]
        assert full, f"lc={lc} has no full-window slot"
        first = full[0]
        rest = sorted((w for w in windows if w is not first),
                      key=lambda d: -d["w"])
        # greedy first-fit into 512-wide bins, `first` seeded at bin0 offset0
        bins = [[]]
        fills = [0]
        for win in [first] + rest:
            placed = False
            for bi in range(len(bins)):
                if fills[bi] + win["w"] <= 512:
                    win = dict(win, cursor=fills[bi], bin=bi)
                    bins[bi].append(win)
                    fills[bi] += win["w"]
                    placed = True
                    break
            if not placed:
                bins.append([dict(win, cursor=0, bin=len(bins))])
                fills.append(win["w"])
        # pair bins into groups of up to 2 (one [128, <=1024] psum tile)
        groups = []
        for g0 in range(0, len(bins), 2):
            slots = []
            for bi in range(g0, min(g0 + 2, len(bins))):
                for win in bins[bi]:
                    slots.append({
                        "sb": win["sb"], "lmin": win["lmin"], "w": win["w"],
                        "goff": (bi - g0) * 512 + win["cursor"],
                    })
            span = max(s2["goff"] + s2["w"] for s2 in slots)
            groups.append({"slots": slots, "span": span})
        plan.append(groups)
        for g in groups:
            t = np.zeros((128, 1024), dtype=np.float32)
            for s2 in g["slots"]:
                sb, lmin, w, goff = s2["sb"], s2["lmin"], s2["w"], s2["goff"]
                t[:, goff:goff + w] = mT[sb * 128:(sb + 1) * 128,
                                         lc * LCW + lmin:lc * LCW + lmin + w]
            packed_tiles.append(t)

    # verification: reassembled coverage == mT exactly (each nonzero in
    # exactly one window, holes zero)
    cover = np.zeros_like(mT)
    gi = 0
    for lc in range(N_LC):
        for g in plan[lc]:
            for s2 in g["slots"]:
                sb, lmin, w, goff = s2["sb"], s2["lmin"], s2["w"], s2["goff"]
                cover[sb * 128:(sb + 1) * 128,
                      lc * LCW + lmin:lc * LCW + lmin + w] = \
                    packed_tiles[gi][:, goff:goff + w]
            gi += 1
    assert np.array_equal(cover, mT), "mask packing is wrong"

    mask_packed = np.stack(packed_tiles).astype(np.float16)
    return plan, mask_packed


_PLAN, _MASK_PACKED = _build_tables()
_NG = _MASK_PACKED.shape[0]          # total groups per pair (mask shared)


# ---------------------------------------------------------------- bass kernel
def _build_nc():
    nc = bacc.Bacc("TRN2", target_bir_lowering=False, debug=False)

    # q/k are pre-cast to fp16, pair-concatenated AND transposed host-side:
    # [128, L] where partitions 0:64 = pair0's E dims, 64:128 = pair1's.
    # Loading them is then a plain contiguous DMA (the device-side xbar
    # DMA-transpose measured ~5us per tensor plus deep queue serialization).
    q_d = nc.dram_tensor("q", [2 * E, L], F16, kind="ExternalInput").ap()
    k_d = nc.dram_tensor("k", [2 * E, S], F16, kind="ExternalInput").ap()
    v_d = nc.dram_tensor("v", [PAIRS_PER_CORE, S, D], F16,
                         kind="ExternalInput").ap()
    mask_d = nc.dram_tensor("mask_packed", [_NG, 128, 1024], F16,
                            kind="ExternalInput").ap()
    o_d = nc.dram_tensor("o", [PAIRS_PER_CORE, L, D], F32,
                         kind="ExternalOutput").ap()

    EXP = mybir.ActivationFunctionType.Exp

    with tile.TileContext(nc) as tc:
        with (
            tc.tile_pool(name="const", bufs=1) as const_pool,
            tc.tile_pool(name="sb", bufs=1) as sb,
            tc.tile_pool(name="pp", bufs=6) as pp,
            tc.tile_pool(name="ob", bufs=2) as ob,
            tc.tile_pool(name="qk_ps", bufs=2, space="PSUM") as qk_ps,
            tc.tile_pool(name="o_ps", bufs=2, space="PSUM") as o_ps,
            tc.tile_pool(name="misc_ps", bufs=1, space="PSUM") as misc_ps,
        ):
            # Q^T / K^T via hardware DMA transpose (fp16), pair-packed on
            # partitions (pair0 -> 0:64, pair1 -> 64:128). Split in halves and
            # ordered by first consumption (early l-chunks only touch the low
            # s/l columns), all on the sync queue in that order.
            qT = const_pool.tile([128, L], F16)
            kT = const_pool.tile([128, S], F16)
            mask_sb = const_pool.tile([128, _NG, 1024], F16)
            v_sb = [const_pool.tile([128, N_SB, D + 1], F16, name=f"v_sb{p}")
                    for p in range(PAIRS_PER_CORE)]

            nc.sync.dma_start(kT, k_d)
            nc.sync.dma_start(qT, q_d)
            nc.sync.dma_start(v_sb[0][:, :, 0:D],
                              v_d[0].rearrange("(t p) d -> p t d", p=128))
            nc.sync.dma_start(v_sb[1][:, :, 0:D],
                              v_d[1].rearrange("(t p) d -> p t d", p=128))
            for a, b in ((0, 3), (3, 8), (8, _NG)):
                nc.sync.dma_start(mask_sb[:, a:b, :],
                                  mask_d[a:b].rearrange("g p c -> p g c"))
            for p in range(PAIRS_PER_CORE):
                nc.vector.memset(v_sb[p][:, :, D], 1.0)

            ident = const_pool.tile([128, 128], F32)
            cmasks.make_identity(nc, ident)

            # zero the two QK psum group slots (4 banks) once with PE matmuls
            # so exp() of packing holes sees exp(0), never leftover inf/nan.
            # PE engine order serializes these against all later PSUM writers.
            zrow = const_pool.tile([1, 512], F16)
            nc.vector.memset(zrow, 0.0)
            for z in range(2):
                zt = qk_ps.tile([128, 1024], F32, tag="s_ps", name=f"zt{z}")
                for b in range(2):
                    nc.tensor.matmul(zt[:, b * 512:(b + 1) * 512],
                                     zrow[:, 0:128], zrow,
                                     start=True, stop=True)

            # main loop
            for p in range(PAIRS_PER_CORE):
                bp = 64 * p
                out_all = sb.tile([128, N_LC * 2, 65], F32, name=f"out_all{p}")
                for lcp in (range(N_LC // 2) if p == 0
                            else list(range(N_LC // 2 - 1, -1, -1))):
                    o_acc = o_ps.tile([65, 2 * LCW], F32, tag="o_acc")
                    for half in range(2):
                        lc = 2 * lcp + half
                        av_list = []
                        for gi, grp in enumerate(_PLAN[lc]):
                            g_idx = sum(len(_PLAN[x]) for x in range(lc)) + gi
                            span = grp["span"]
                            s_ps = qk_ps.tile([128, 1024], F32, tag="s_ps")
                            for s2 in grp["slots"]:
                                sb_, lmin, w, goff = (s2["sb"], s2["lmin"],
                                                      s2["w"], s2["goff"])
                                nc.tensor.matmul(
                                    s_ps[:, goff:goff + w],
                                    kT[bp:bp + 64, sb_ * 128:(sb_ + 1) * 128],
                                    qT[bp:bp + 64,
                                       lc * LCW + lmin:lc * LCW + lmin + w],
                                    start=True, stop=True)
                            p_t = pp.tile([128, 1024], F16, tag="p_t")
                            nc.scalar.activation(p_t[:, :span], s_ps[:, :span],
                                                 EXP, scale=SCALE)
                            nc.vector.tensor_mul(p_t[:, :span], p_t[:, :span],
                                                 mask_sb[:, g_idx, :span])
                            av_list.append((grp, p_t))
                        n_av = sum(len(g["slots"]) for g, _ in av_list)
                        ai = 0
                        for grp, p_t in av_list:
                            for s2 in grp["slots"]:
                                sb_, lmin, w, goff = (s2["sb"], s2["lmin"],
                                                      s2["w"], s2["goff"])
                                nc.tensor.matmul(
                                    o_acc[:, half * LCW + lmin:
                                          half * LCW + lmin + w],
                                    v_sb[p][:, sb_, :],
                                    p_t[:, goff:goff + w],
                                    start=(ai == 0), stop=(ai == n_av - 1),
                                    skip_group_check=True)
                                ai += 1
                    # output stage for these two l-chunks (4 l-blocks of 128)
                    o_sb = ob.tile([65, 2 * LCW], F32, tag="o_sb")
                    nc.scalar.copy(o_sb, o_acc)
                    ot = misc_ps.tile([128, 4 * 65], F32, tag="outt")
                    for j in range(4):
                        nc.tensor.transpose(ot[:, j * 65:(j + 1) * 65],
                                            o_sb[:, j * 128:(j + 1) * 128],
                                            ident[0:65, 0:65])
                    nc.vector.tensor_copy(out_all[:, 4 * lcp:4 * lcp + 4, :],
                                          ot.rearrange("p (j c) -> p j c", c=65))
                # normalize and store
                rinv = sb.tile([128, N_LC * 2], F32, name=f"rinv{p}")
                nc.vector.reciprocal(rinv, out_all[:, :, 64])
                o_n = sb.tile([128, N_LC * 2, 64], F32, name=f"o_n{p}")
                nc.vector.tensor_tensor(
                    o_n, out_all[:, :, 0:64],
                    rinv[:, :, None].to_broadcast((128, N_LC * 2, 64)),
                    mybir.AluOpType.mult)
                nc.sync.dma_start(
                    o_d[p].rearrange("(t p) d -> p t d", p=128), o_n)

    nc.finalize()
    return nc


_NC_CACHE = None


def _get_nc():
    global _NC_CACHE
    if _NC_CACHE is None:
        _NC_CACHE = _build_nc()
    return _NC_CACHE


# ------------------------------------------------------------------- wrapper
def kernel(queries, keys, values):
    queries = np.ascontiguousarray(np.asarray(queries, dtype=np.float32))
    keys = np.ascontiguousarray(np.asarray(keys, dtype=np.float32))
    values = np.ascontiguousarray(np.asarray(values, dtype=np.float32))

    in_maps = []
    for c in range(N_CORES):
        qs, ks, vs = [], [], []
        for j in range(PAIRS_PER_CORE):
            i = c * PAIRS_PER_CORE + j
            b, h = divmod(i, H)
            qs.append(queries[b, :, h, :])
            ks.append(keys[b, :, h, :])
            vs.append(values[b, :, h, :])
        in_maps.append({
            "q": np.ascontiguousarray(
                np.concatenate(qs, axis=1).astype(np.float16).T),
            "k": np.ascontiguousarray(
                np.concatenate(ks, axis=1).astype(np.float16).T),
            "v": np.ascontiguousarray(np.stack(vs)).astype(np.float16),
            "mask_packed": _MASK_PACKED,
        })

    res = run_bass_kernel_spmd(_get_nc(), in_maps,
                               core_ids=list(range(N_CORES)))

    out = np.empty((B, L, H, D), dtype=np.float32)
    for c in range(N_CORES):
        for j in range(PAIRS_PER_CORE):
            i = c * PAIRS_PER_CORE + j
            b, h = divmod(i, H)
            out[b, :, h, :] = res.results[c]["o"][j]
    return out


# revision 24
# speedup vs baseline: 1.2603x; 1.0169x over previous
"""Log-sparse attention Trainium2 kernel (self-contained).

Problem: B=2, L=S=2048, H=8, E=D=64 fp32.
  scores = q @ k^T (per b,h); masked log-sparse; A = softmax(scores/8); out = A @ v.

Strategy:
  - Shard the 16 (b,h) pairs across 8 cores, 2 pairs per core (SPMD, same
    program, different data).
  - Everything computed transposed: S^T[s,l] = K^T.T @ Q^T so that the A@V
    matmul (lhsT = V tiles in natural [s,d] layout, rhs = P^T) needs no P
    transposes. Softmax denominator comes from a ones-column appended to V.
    No max-subtraction (scores/8 ~ N(0,1); exp is safe in fp32/fp16 range).
  - The log-sparse mask is static: only 58 of 128 (s_block=128 x l_chunk=256)
    tiles per pair are nonzero, and within those only a trimmed l-window
    matters. Windows are bin-packed into [128, <=1024] PSUM groups (2 banks);
    exp + mask-multiply run once per group.
  - fp16 operands for QK and AV matmuls (1 cyc/row on PE at any moving dim,
    vs 4 for fp32), fp32 PSUM accumulation throughout. Masked positions are
    exact zeros (multiplicative 0/1 mask), matching the reference's
    exp(-1e9/8 - m) == 0 underflow.
"""
import math

import numpy as np
import ml_dtypes

import concourse.bacc as bacc
import concourse.mybir as mybir
import concourse.tile as tile
from concourse import masks as cmasks
from concourse.bass_utils import run_bass_kernel_spmd

F32 = mybir.dt.float32
F16 = mybir.dt.float16

B, L, S, H, E, D = 2, 2048, 2048, 8, 64, 64
N_CORES = 8
PAIRS_PER_CORE = 2
LCW = 256            # l-chunk width
N_LC = L // LCW      # 8
N_SB = S // 128      # 16
ALIGN = 16
SCALE = 1.0 / math.sqrt(E)


# ---------------------------------------------------------------- host tables
def _log_row_mask(index, sub_len, win_len):
    log_l = math.ceil(np.log2(sub_len))
    mask = np.zeros(win_len, dtype=np.float32)
    if (win_len // sub_len) * 2 * log_l > index:
        mask[:index + 1] = 1
    else:
        while index >= 0:
            if index - log_l + 1 < 0:
                mask[:index] = 1
                break
            mask[index - log_l + 1:index + 1] = 1
            for i in range(0, log_l):
                new_index = index - log_l + 1 - 2 ** i
                if (index - new_index) <= sub_len and new_index >= 0:
                    mask[new_index] = 1
            index -= sub_len
    return mask


def _build_tables():
    """Returns (plan, mask_packed).

    plan[lc] = list of groups; each group is a list of slot dicts
      {sb, lmin, w, goff} -- goff is the column offset inside the [128, gspan]
      psum group tile. Each bin occupies a 512-aligned half so no matmul
      window crosses a PSUM bank boundary.
    mask_packed: [NG, 128, 1024] float16, zeros in packing holes.
    """
    m = np.stack([_log_row_mask(i, S, L) for i in range(L)])  # [l, s]
    mT = m.T  # [s, l]

    plan = []
    packed_tiles = []
    for lc in range(N_LC):
        windows = []
        for sb in range(N_SB):
            sub = mT[sb * 128:(sb + 1) * 128, lc * LCW:(lc + 1) * LCW]
            cols = np.where(sub.any(axis=0))[0]
            if len(cols) == 0:
                continue
            lmin = int(cols.min() // ALIGN) * ALIGN
            lmax = int(np.ceil((cols.max() + 1) / ALIGN) * ALIGN)
            windows.append({"sb": sb, "lmin": lmin, "w": lmax - lmin})
        # the slot covering the whole chunk must exist (first AV write per lc
        # must cover all 256 columns so start=True initializes the region)
        full = [w for w in windows if w["lmin"] == 0 and w["w"] == LCW]
        assert full, f"lc={lc} has no full-window slot"
        first = full[0]
        rest = sorted((w for w in windows if w is not first),
                      key=lambda d: -d["w"])
        # greedy first-fit into 512-wide bins, `first` seeded at bin0 offset0
        bins = [[]]
        fills = [0]
        for win in [first] + rest:
            placed = False
            for bi in range(len(bins)):
                if fills[bi] + win["w"] <= 512:
                    win = dict(win, cursor=fills[bi], bin=bi)
                    bins[bi].append(win)
                    fills[bi] += win["w"]
                    placed = True
                    break
            if not placed:
                bins.append([dict(win, cursor=0, bin=len(bins))])
                fills.append(win["w"])
        # pair bins into groups of up to 2 (one [128, <=1024] psum tile)
        groups = []
        for g0 in range(0, len(bins), 2):
            slots = []
            for bi in range(g0, min(g0 + 2, len(bins))):
                for win in bins[bi]:
                    slots.append({
                        "sb": win["sb"], "lmin": win["lmin"], "w": win["w"],
                        "goff": (bi - g0) * 512 + win["cursor"],
                    })
            span = max(s2["goff"] + s2["w"] for s2 in slots)
            groups.append({"slots": slots, "span": span})
        plan.append(groups)
        for g in groups:
            t = np.zeros((128, 1024), dtype=np.float32)
            for s2 in g["slots"]:
                sb, lmin, w, goff = s2["sb"], s2["lmin"], s2["w"], s2["goff"]
                t[:, goff:goff + w] = mT[sb * 128:(sb + 1) * 128,
                                         lc * LCW + lmin:lc * LCW + lmin + w]
            packed_tiles.append(t)

    # verification: reassembled coverage == mT exactly (each nonzero in
    # exactly one window, holes zero)
    cover = np.zeros_like(mT)
    gi = 0
    for lc in range(N_LC):
        for g in plan[lc]:
            for s2 in g["slots"]:
                sb, lmin, w, goff = s2["sb"], s2["lmin"], s2["w"], s2["goff"]
                cover[sb * 128:(sb + 1) * 128,
                      lc * LCW + lmin:lc * LCW + lmin + w] = \
                    packed_tiles[gi][:, goff:goff + w]
            gi += 1
    assert np.array_equal(cover, mT), "mask packing is wrong"

    mask_packed = np.stack(packed_tiles).astype(np.float16)
    return plan, mask_packed


_PLAN, _MASK_PACKED = _build_tables()
_NG = _MASK_PACKED.shape[0]          # total groups per pair (mask shared)


# ---------------------------------------------------------------- bass kernel
def _build_nc():
    nc = bacc.Bacc("TRN2", target_bir_lowering=False, debug=False)

    # q/k are pre-cast to fp16, pair-concatenated AND transposed host-side:
    # [128, L] where partitions 0:64 = pair0's E dims, 64:128 = pair1's.
    # Loading them is then a plain contiguous DMA (the device-side xbar
    # DMA-transpose measured ~5us per tensor plus deep queue serialization).
    q_d = nc.dram_tensor("q", [2 * E, L], F16, kind="ExternalInput").ap()
    k_d = nc.dram_tensor("k", [2 * E, S], F16, kind="ExternalInput").ap()
    v_d = nc.dram_tensor("v", [PAIRS_PER_CORE, S, D], F16,
                         kind="ExternalInput").ap()
    mask_d = nc.dram_tensor("mask_packed", [_NG, 128, 1024], F16,
                            kind="ExternalInput").ap()
    o_d = nc.dram_tensor("o", [PAIRS_PER_CORE, L, D], F32,
                         kind="ExternalOutput").ap()

    EXP = mybir.ActivationFunctionType.Exp

    with tile.TileContext(nc) as tc:
        with (
            tc.tile_pool(name="const", bufs=1) as const_pool,
            tc.tile_pool(name="sb", bufs=1) as sb,
            tc.tile_pool(name="pp", bufs=6) as pp,
            tc.tile_pool(name="ob", bufs=2) as ob,
            tc.tile_pool(name="qk_ps", bufs=2, space="PSUM") as qk_ps,
            tc.tile_pool(name="o_ps", bufs=2, space="PSUM") as o_ps,
            tc.tile_pool(name="misc_ps", bufs=1, space="PSUM") as misc_ps,
        ):
            # Q^T / K^T via hardware DMA transpose (fp16), pair-packed on
            # partitions (pair0 -> 0:64, pair1 -> 64:128). Split in halves and
            # ordered by first consumption (early l-chunks only touch the low
            # s/l columns), all on the sync queue in that order.
            qT = const_pool.tile([128, L], F16)
            kT = const_pool.tile([128, S], F16)
            mask_sb = const_pool.tile([128, _NG, 1024], F16)
            v_sb = [const_pool.tile([128, N_SB, D + 1], F16, name=f"v_sb{p}")
                    for p in range(PAIRS_PER_CORE)]

            nc.sync.dma_start(kT, k_d)
            nc.sync.dma_start(qT, q_d)
            nc.sync.dma_start(mask_sb[:, 0:3, :],
                              mask_d[0:3].rearrange("g p c -> p g c"))
            nc.sync.dma_start(v_sb[0][:, :, 0:D],
                              v_d[0].rearrange("(t p) d -> p t d", p=128))
            nc.sync.dma_start(mask_sb[:, 3:8, :],
                              mask_d[3:8].rearrange("g p c -> p g c"))
            nc.sync.dma_start(v_sb[1][:, :, 0:D],
                              v_d[1].rearrange("(t p) d -> p t d", p=128))
            nc.sync.dma_start(mask_sb[:, 8:_NG, :],
                              mask_d[8:_NG].rearrange("g p c -> p g c"))
            for p in range(PAIRS_PER_CORE):
                nc.vector.memset(v_sb[p][:, :, D], 1.0)

            ident = const_pool.tile([128, 128], F16)
            cmasks.make_identity(nc, ident)

            # zero the two QK psum group slots (4 banks) once with PE matmuls
            # so exp() of packing holes sees exp(0), never leftover inf/nan.
            # PE engine order serializes these against all later PSUM writers.
            zrow = const_pool.tile([1, 512], F16)
            nc.vector.memset(zrow, 0.0)
            for z in range(2):
                zt = qk_ps.tile([128, 1024], F32, tag="s_ps", name=f"zt{z}")
                for b in range(2):
                    nc.tensor.matmul(zt[:, b * 512:(b + 1) * 512],
                                     zrow[:, 0:128], zrow,
                                     start=True, stop=True)

            # main loop
            for p in range(PAIRS_PER_CORE):
                bp = 64 * p
                out_all = sb.tile([128, N_LC * 2, 65], F32, name=f"out_all{p}")
                for lcp in (range(N_LC // 2) if p == 0
                            else list(range(N_LC // 2 - 1, -1, -1))):
                    o_acc = o_ps.tile([65, 2 * LCW], F32, tag="o_acc")
                    for half in range(2):
                        lc = 2 * lcp + half
                        av_list = []
                        for gi, grp in enumerate(_PLAN[lc]):
                            g_idx = sum(len(_PLAN[x]) for x in range(lc)) + gi
                            span = grp["span"]
                            s_ps = qk_ps.tile([128, 1024], F32, tag="s_ps")
                            for s2 in grp["slots"]:
                                sb_, lmin, w, goff = (s2["sb"], s2["lmin"],
                                                      s2["w"], s2["goff"])
                                nc.tensor.matmul(
                                    s_ps[:, goff:goff + w],
                                    kT[bp:bp + 64, sb_ * 128:(sb_ + 1) * 128],
                                    qT[bp:bp + 64,
                                       lc * LCW + lmin:lc * LCW + lmin + w],
                                    start=True, stop=True)
                            p_t = pp.tile([128, 1024], F16, tag="p_t")
                            nc.scalar.activation(p_t[:, :span], s_ps[:, :span],
                                                 EXP, scale=SCALE)
                            nc.vector.tensor_mul(p_t[:, :span], p_t[:, :span],
                                                 mask_sb[:, g_idx, :span])
                            av_list.append((grp, p_t))
                        n_av = sum(len(g["slots"]) for g, _ in av_list)
                        ai = 0
                        for grp, p_t in av_list:
                            for s2 in grp["slots"]:
                                sb_, lmin, w, goff = (s2["sb"], s2["lmin"],
                                                      s2["w"], s2["goff"])
                                nc.tensor.matmul(
                                    o_acc[:, half * LCW + lmin:
                                          half * LCW + lmin + w],
                                    v_sb[p][:, sb_, :],
                                    p_t[:, goff:goff + w],
                                    start=(ai == 0), stop=(ai == n_av - 1),
                                    skip_group_check=True)
                                ai += 1
                    # output stage for these two l-chunks (4 l-blocks of 128)
                    o_sb = ob.tile([65, 2 * LCW], F16, tag="o_sb")
                    nc.scalar.copy(o_sb, o_acc)
                    ot = misc_ps.tile([128, 4, 66], F16, tag="outt")
                    for j in range(4):
                        nc.tensor.transpose(ot[:, j, 0:65],
                                            o_sb[:, j * 128:(j + 1) * 128],
                                            ident[0:65, 0:65])
                    nc.vector.tensor_copy(out_all[:, 4 * lcp:4 * lcp + 4, :],
                                          ot[:, :, 0:65])
                # normalize and store
                rinv = sb.tile([128, N_LC * 2], F32, name=f"rinv{p}")
                nc.vector.reciprocal(rinv, out_all[:, :, 64])
                o_n = sb.tile([128, N_LC * 2, 64], F32, name=f"o_n{p}")
                nc.vector.tensor_tensor(
                    o_n, out_all[:, :, 0:64],
                    rinv[:, :, None].to_broadcast((128, N_LC * 2, 64)),
                    mybir.AluOpType.mult)
                nc.sync.dma_start(
                    o_d[p].rearrange("(t p) d -> p t d", p=128), o_n)

    nc.finalize()
    return nc


_NC_CACHE = None


def _get_nc():
    global _NC_CACHE
    if _NC_CACHE is None:
        _NC_CACHE = _build_nc()
    return _NC_CACHE


# ------------------------------------------------------------------- wrapper
def kernel(queries, keys, values):
    queries = np.ascontiguousarray(np.asarray(queries, dtype=np.float32))
    keys = np.ascontiguousarray(np.asarray(keys, dtype=np.float32))
    values = np.ascontiguousarray(np.asarray(values, dtype=np.float32))

    in_maps = []
    for c in range(N_CORES):
        qs, ks, vs = [], [], []
        for j in range(PAIRS_PER_CORE):
            i = c * PAIRS_PER_CORE + j
            b, h = divmod(i, H)
            qs.append(queries[b, :, h, :])
            ks.append(keys[b, :, h, :])
            vs.append(values[b, :, h, :])
        in_maps.append({
            "q": np.ascontiguousarray(
                np.concatenate(qs, axis=1).astype(np.float16).T),
            "k": np.ascontiguousarray(
                np.concatenate(ks, axis=1).astype(np.float16).T),
            "v": np.ascontiguousarray(np.stack(vs)).astype(np.float16),
            "mask_packed": _MASK_PACKED,
        })

    res = run_bass_kernel_spmd(_get_nc(), in_maps,
                               core_ids=list(range(N_CORES)))

    out = np.empty((B, L, H, D), dtype=np.float32)
    for c in range(N_CORES):
        for j in range(PAIRS_PER_CORE):
            i = c * PAIRS_PER_CORE + j
            b, h = divmod(i, H)
            out[b, :, h, :] = res.results[c]["o"][j]
    return out
